# revision 13
# baseline (speedup 1.0000x reference)
"""Trainium2 Bass kernel for nn_Epipolar (epipolar max-sampling).

Strategy
--------
out[n,c,h,w] = max_s bilinear(feat2[n,:,:,:], loc(s, n, h, w))

* Host: computes the 3x3 fundamental matrices F[n] (eigh/pinv of the tiny
  3x4 camera matrices, 24 input floats) and re-lays-out feat2 into a padded
  interleaved lookup table T[n][(y+1)*66+(x+1)] = [g, dx, dy, dxy] per
  channel, where g is the pixel value and dx/dy/dxy are forward differences.
  With that table a bilinear sample is g + wx*dx + wy*dy + wx*wy*dxy with a
  single gathered row per sample.
* Device (8 NeuronCores, data-parallel over output rows; core k owns output
  rows [8k, 8k+8) of both batch elements — outputs are disjoint, no
  collectives):
    - per-pixel epipolar line coefficients  l = F @ [x, y, 1]
    - border intersections, validity masks, stable pick-first-2 selection
    - 64 sample positions per pixel, bilinear weights + table indices
    - indirect-DMA gather of 1024-float table rows (128 pixels x 8 samples
      per chunk), fused scalar_tensor_tensor bilinear, running max
    - PE transpose [pixel, channel] -> [channel, pixel] and DMA out.
"""

import numpy as np

# ---------------------------------------------------------------------------
# Problem constants (hardcoded per contract)
N, C, H, W = 2, 256, 64, 64
S = 64
EPS = 1e-3
NCORES = 8
ROWS_PER_CORE = H // NCORES          # 8
PIX_PER_N = ROWS_PER_CORE * W        # 512 pixels per batch element per core
NCOL = PIX_PER_N // 128              # 4 partition-blocks of 128 pixels
GP = 66                              # padded grid is 66x66 (coords -1..64)
NQ = GP * GP                         # 4356 table rows
SCH = 16                             # samples per gather chunk
NCH = S // SCH                       # chunks per (n, col)
CLIPHI = np.float32(64.999996)       # just under 65, keeps x0 <= 64

TABLE_DT = "float32"                 # table dtype: float32 (safe) / float16


def _find_fundamental_host(P1, P2):
    """Exact replica of reference.find_fundamental, on jax CPU (matches the
    harness's own f32 eigh/pinv numerics)."""
    import jax

    with jax.default_device(jax.devices("cpu")[0]):
        import jax.numpy as jnp

        P1j = jnp.asarray(P1, jnp.float32)
        P2j = jnp.asarray(P2, jnp.float32)
        M = jnp.einsum("nij,nik->njk", P1j, P1j)
        _, V = jnp.linalg.eigh(M)
        Cc = V[..., 0]
        e2 = jnp.einsum("nij,nj->ni", P2j, Cc)
        z = jnp.zeros_like(e2[:, 0])
        skew = jnp.stack(
            [
                jnp.stack([z, -e2[:, 2], e2[:, 1]], -1),
                jnp.stack([e2[:, 2], z, -e2[:, 0]], -1),
                jnp.stack([-e2[:, 1], e2[:, 0], z], -1),
            ],
            -2,
        )
        F = skew @ P2j @ jnp.linalg.pinv(P1j)
        return np.asarray(F, np.float32)


def _build_tables(feat2, np_dt):
    """[N, NQ, 4*C] interleaved (g, dx, dy, dxy) over the padded 66x66 grid."""
    pad = np.zeros((N, H + 3, W + 3, C), np.float32)
    pad[:, 1 : H + 1, 1 : W + 1, :] = feat2.transpose(0, 2, 3, 1)
    g = pad[:, 0:GP, 0:GP, :]
    r = pad[:, 0:GP, 1 : GP + 1, :]
    d = pad[:, 1 : GP + 1, 0:GP, :]
    q = pad[:, 1 : GP + 1, 1 : GP + 1, :]
    T = np.concatenate([g, r - g, d - g, q - r - d + g], axis=-1)  # N,66,66,4C
    return np.ascontiguousarray(T.reshape(N, NQ, 4 * C).astype(np_dt))


# ---------------------------------------------------------------------------
# Bass program
# ---------------------------------------------------------------------------

def build_program(table_dt_name=TABLE_DT):
    import concourse.bacc as bacc
    import concourse.bass as bass
    import concourse.mybir as mybir
    import concourse.tile as tile
    from concourse.masks import make_identity

    dt = mybir.dt
    op = mybir.AluOpType
    TD = getattr(dt, table_dt_name)

    nc = bacc.Bacc("TRN2", target_bir_lowering=False, debug=False)

    tabs = [
        nc.dram_tensor(f"table{n}", [NQ, 4 * C], TD, kind="ExternalInput").ap()
        for n in range(N)
    ]
    fmat = nc.dram_tensor("fmat", [128, 16 * N], dt.float32, kind="ExternalInput").ap()
    xw_in = nc.dram_tensor("xw", [128, NCOL], dt.float32, kind="ExternalInput").ap()
    yh_in = nc.dram_tensor("yh", [128, NCOL], dt.float32, kind="ExternalInput").ap()
    trow_in = nc.dram_tensor("trow", [128, S], dt.float32, kind="ExternalInput").ap()
    y_out = nc.dram_tensor(
        "y", [N, C, ROWS_PER_CORE, W], dt.float32, kind="ExternalOutput"
    ).ap()

    with tile.TileContext(nc) as tc:
        with (
            tc.tile_pool(name="const", bufs=1) as cpool,
            tc.tile_pool(name="geo", bufs=1) as gpool,
            tc.tile_pool(name="pers", bufs=1) as ppool,
            tc.tile_pool(name="work", bufs=2) as wpool,
            tc.tile_pool(name="acc", bufs=2) as apool,
            tc.tile_pool(name="psum", bufs=2, space="PSUM") as pspool,
        ):
            # ---- constants to SBUF
            fm = cpool.tile([128, 16 * N], dt.float32, tag="fm", name="fm")
            xw = cpool.tile([128, NCOL], dt.float32, tag="xw", name="xw")
            yh = cpool.tile([128, NCOL], dt.float32, tag="yh", name="yh")
            trow = cpool.tile([128, S], dt.float32, tag="trow", name="trow")
            ident = cpool.tile([128, 128], dt.float32, tag="ident", name="ident")
            nc.sync.dma_start(out=fm[:], in_=fmat[:])
            nc.sync.dma_start(out=xw[:], in_=xw_in[:])
            nc.sync.dma_start(out=yh[:], in_=yh_in[:])
            nc.sync.dma_start(out=trow[:], in_=trow_in[:])
            make_identity(nc, ident[:])

            def F(n, i):
                return fm[:, 16 * n + i : 16 * n + i + 1]

            P8 = [128, N * NCOL]  # geometry tiles cover both batches

            def gt(tag):
                return gpool.tile(P8, dt.float32, tag=tag, name=tag)

            a_t, b_t, c_t = gt("a"), gt("b"), gt("c")
            tmp, tmp2, tmp3 = gt("tmp"), gt("tmp2"), gt("tmp3")

            # line coefficients per batch half, replicating XLA's einsum
            # rounding exactly: abc = fl(fma(F1, y, fl(F0*x)) + F2).
            # y is a small integer, so F1*y = F1hi*y + F1lo*y with both
            # products exact; the fma is then an exactly-rounded 3-term sum
            # via TwoSum.
            s_t, bb_t, u1_t, u2_t, u3_t, e_t2 = (
                gt("s_t"), gt("bb_t"), gt("u1_t"), gt("u2_t"), gt("u3_t"),
                gt("e_t2"),
            )
            for n in range(N):
                sl = slice(n * NCOL, (n + 1) * NCOL)
                for dst, r0 in ((a_t, 0), (b_t, 1), (c_t, 2)):
                    c0 = tmp[:, sl]
                    t1 = tmp2[:, sl]
                    t2 = tmp3[:, sl]
                    nc.vector.tensor_scalar(
                        out=c0, in0=xw[:], scalar1=F(n, 4 * r0 + 0),
                        scalar2=None, op0=op.mult,
                    )
                    nc.vector.tensor_scalar(
                        out=t1, in0=yh[:], scalar1=F(n, 4 * r0 + 1),
                        scalar2=None, op0=op.mult,
                    )
                    nc.vector.tensor_scalar(
                        out=t2, in0=yh[:], scalar1=F(n, 4 * r0 + 2),
                        scalar2=None, op0=op.mult,
                    )
                    s, bb, u1, u2, u3, e2s = (
                        s_t[:, sl], bb_t[:, sl], u1_t[:, sl], u2_t[:, sl],
                        u3_t[:, sl], e_t2[:, sl],
                    )
                    nc.vector.tensor_tensor(out=s, in0=t1, in1=c0, op=op.add)
                    nc.vector.tensor_tensor(out=bb, in0=s, in1=t1, op=op.subtract)
                    nc.vector.tensor_tensor(out=u1, in0=s, in1=bb, op=op.subtract)
                    nc.vector.tensor_tensor(out=u2, in0=t1, in1=u1, op=op.subtract)
                    nc.vector.tensor_tensor(out=u3, in0=c0, in1=bb, op=op.subtract)
                    nc.vector.tensor_tensor(out=e2s, in0=u2, in1=u3, op=op.add)
                    nc.vector.tensor_tensor(out=e2s, in0=e2s, in1=t2, op=op.add)
                    nc.vector.tensor_tensor(out=s, in0=s, in1=e2s, op=op.add)
                    nc.vector.tensor_scalar(
                        out=dst[:, sl], in0=s, scalar1=F(n, 4 * r0 + 3),
                        scalar2=None, op0=op.add,
                    )

            negc = gt("negc")
            nc.vector.tensor_scalar(
                out=negc[:], in0=c_t[:], scalar1=-1.0, scalar2=None, op0=op.mult
            )

            # db = sign(b)*max(|b|, EPS), true divide (matches reference)
            db_t, da_t = gt("db_t"), gt("da_t")
            for srct, dst in ((b_t, db_t), (a_t, da_t)):
                nc.scalar.activation(
                    out=tmp[:], in_=srct[:], func=mybir.ActivationFunctionType.Abs
                )
                nc.vector.tensor_scalar(
                    out=tmp[:], in0=tmp[:], scalar1=float(EPS), scalar2=None,
                    op0=op.max,
                )
                nc.scalar.activation(
                    out=tmp3[:], in_=srct[:], func=mybir.ActivationFunctionType.Sign
                )
                nc.vector.tensor_tensor(
                    out=dst[:], in0=tmp3[:], in1=tmp[:], op=op.mult
                )

            rdb, rda = gt("rdb"), gt("rda")
            nc.vector.reciprocal(out=rdb[:], in_=db_t[:])
            nc.vector.reciprocal(out=rda[:], in_=da_t[:])

            by1, by2, bx0, bx3 = gt("by1"), gt("by2"), gt("bx0"), gt("bx3")
            nc.vector.tensor_tensor(out=by1[:], in0=negc[:], in1=rdb[:], op=op.mult)
            nc.vector.scalar_tensor_tensor(
                out=tmp[:], in0=a_t[:], scalar=-63.0, in1=negc[:],
                op0=op.mult, op1=op.add,
            )
            nc.vector.tensor_tensor(out=by2[:], in0=tmp[:], in1=rdb[:], op=op.mult)
            nc.vector.tensor_tensor(out=bx0[:], in0=negc[:], in1=rda[:], op=op.mult)
            nc.vector.scalar_tensor_tensor(
                out=tmp[:], in0=b_t[:], scalar=-63.0, in1=negc[:],
                op0=op.mult, op1=op.add,
            )
            nc.vector.tensor_tensor(out=bx3[:], in0=tmp[:], in1=rda[:], op=op.mult)

            # validity masks of the four border intersections
            masks = [gt(f"m{i}") for i in range(4)]
            mspec = [
                (bx0, op.is_ge, op.is_lt),
                (by1, op.is_gt, op.is_le),
                (by2, op.is_ge, op.is_lt),
                (bx3, op.is_gt, op.is_le),
            ]
            for mi, (src, lo_op, hi_op) in zip(masks, mspec):
                nc.vector.tensor_scalar(
                    out=tmp[:], in0=src[:], scalar1=float(EPS), scalar2=None, op0=lo_op
                )
                nc.vector.tensor_scalar(
                    out=tmp2[:], in0=src[:], scalar1=float(63.0 - EPS), scalar2=None,
                    op0=hi_op,
                )
                nc.vector.tensor_tensor(
                    out=mi[:], in0=tmp[:], in1=tmp2[:], op=op.mult
                )

            nint = gt("nint")
            nc.vector.tensor_tensor(out=nint[:], in0=masks[0][:], in1=masks[1][:], op=op.add)
            nc.vector.tensor_tensor(out=nint[:], in0=nint[:], in1=masks[2][:], op=op.add)
            nc.vector.tensor_tensor(out=nint[:], in0=nint[:], in1=masks[3][:], op=op.add)
            valid2 = gt("valid2")
            nc.vector.tensor_scalar(
                out=valid2[:], in0=nint[:], scalar1=2.0, scalar2=None, op0=op.is_ge
            )

            # tmp_mask: where(valid2, mask, [1,1,0,0])
            tmask = [gt(f"tm{i}") for i in range(4)]
            for i in range(4):
                if i < 2:
                    nc.vector.tensor_scalar(
                        out=tmp[:], in0=masks[i][:], scalar1=-1.0, scalar2=None,
                        op0=op.add,
                    )
                    nc.vector.tensor_tensor(
                        out=tmp2[:], in0=tmp[:], in1=valid2[:], op=op.mult
                    )
                    nc.vector.tensor_scalar(
                        out=tmask[i][:], in0=tmp2[:], scalar1=1.0, scalar2=None,
                        op0=op.add,
                    )
                else:
                    nc.vector.tensor_tensor(
                        out=tmask[i][:], in0=masks[i][:], in1=valid2[:], op=op.mult
                    )

            # keys + two smallest (keys are distinct by construction)
            keys = [gt(f"k{i}") for i in range(4)]
            for i in range(4):
                nc.vector.tensor_scalar(
                    out=keys[i][:], in0=tmask[i][:], scalar1=-4.0,
                    scalar2=float(4 + i), op0=op.mult, op1=op.add,
                )
            kmin = gt("kmin")
            nc.vector.tensor_tensor(out=kmin[:], in0=keys[0][:], in1=keys[1][:], op=op.min)
            nc.vector.tensor_tensor(out=kmin[:], in0=kmin[:], in1=keys[2][:], op=op.min)
            nc.vector.tensor_tensor(out=kmin[:], in0=kmin[:], in1=keys[3][:], op=op.min)
            e_sel = [gt(f"e{i}") for i in range(4)]
            for i in range(4):
                nc.vector.tensor_tensor(
                    out=e_sel[i][:], in0=keys[i][:], in1=kmin[:], op=op.is_equal
                )
            kmin2 = gt("kmin2")
            k2t = [gt(f"k2t{i}") for i in range(4)]
            for i in range(4):
                nc.vector.scalar_tensor_tensor(
                    out=k2t[i][:], in0=e_sel[i][:], scalar=100.0, in1=keys[i][:],
                    op0=op.mult, op1=op.add,
                )
            nc.vector.tensor_tensor(out=kmin2[:], in0=k2t[0][:], in1=k2t[1][:], op=op.min)
            nc.vector.tensor_tensor(out=kmin2[:], in0=kmin2[:], in1=k2t[2][:], op=op.min)
            nc.vector.tensor_tensor(out=kmin2[:], in0=kmin2[:], in1=k2t[3][:], op=op.min)
            f_sel = [gt(f"f{i}") for i in range(4)]
            for i in range(4):
                nc.vector.tensor_tensor(
                    out=f_sel[i][:], in0=keys[i][:], in1=kmin2[:], op=op.is_equal
                )

            # selected endpoint coordinates
            # point coords: p0=(bx0,0) p1=(0,by1) p2=(63,by2) p3=(bx3,63)
            def sel_xy(sel, xo, yo):
                # x = s0*bx0 + s2*63 + s3*bx3
                nc.vector.tensor_tensor(out=xo[:], in0=sel[0][:], in1=bx0[:], op=op.mult)
                nc.vector.scalar_tensor_tensor(
                    out=xo[:], in0=sel[2][:], scalar=63.0, in1=xo[:],
                    op0=op.mult, op1=op.add,
                )
                nc.vector.tensor_tensor(out=tmp[:], in0=sel[3][:], in1=bx3[:], op=op.mult)
                nc.vector.tensor_tensor(out=xo[:], in0=xo[:], in1=tmp[:], op=op.add)
                # y = s1*by1 + s2*by2 + s3*63
                nc.vector.tensor_tensor(out=yo[:], in0=sel[1][:], in1=by1[:], op=op.mult)
                nc.vector.tensor_tensor(out=tmp[:], in0=sel[2][:], in1=by2[:], op=op.mult)
                nc.vector.tensor_tensor(out=yo[:], in0=yo[:], in1=tmp[:], op=op.add)
                nc.vector.scalar_tensor_tensor(
                    out=yo[:], in0=sel[3][:], scalar=63.0, in1=yo[:],
                    op0=op.mult, op1=op.add,
                )

            p1x, p1y, p2x, p2y = gt("p1x"), gt("p1y"), gt("p2x"), gt("p2y")
            sel_xy(e_sel, p1x, p1y)
            sel_xy(f_sel, p2x, p2y)

            # invalid pixels -> both endpoints at -10000 (exact select)
            inval = gpool.tile(P8, dt.uint8, tag="inval", name="inval")
            nc.vector.tensor_scalar(
                out=inval[:], in0=nint[:], scalar1=2.0, scalar2=None, op0=op.is_lt
            )
            neg1e4 = gt("neg1e4")
            nc.vector.memset(neg1e4[:], -10000.0)
            for t in (p1x, p1y, p2x, p2y):
                nc.vector.copy_predicated(out=t[:], mask=inval[:], data=neg1e4[:])
            vecx, vecy = gt("vecx"), gt("vecy")
            nc.vector.tensor_tensor(out=vecx[:], in0=p2x[:], in1=p1x[:], op=op.subtract)
            nc.vector.tensor_tensor(out=vecy[:], in0=p2y[:], in1=p1y[:], op=op.subtract)

            # ---- per (n, col): sample expansion, weights, indices,
            #      then immediately the gather + bilinear + max for that block
            trow_b = trow[:]
            for n in range(N):
                for col in range(NCOL):
                    gcol = n * NCOL + col
                    key = (n, col)
                    wx_t = ppool.tile([128, S], dt.float32, tag="wx", name="wx", bufs=2)
                    wy_t = ppool.tile([128, S], dt.float32, tag="wy", name="wy", bufs=2)
                    wxy_t = ppool.tile([128, S], dt.float32, tag="wxy", name="wxy", bufs=2)
                    qi_t = ppool.tile([128, S], dt.int32, tag="qi", name="qi", bufs=2)

                    qf = gpool.tile([128, S], dt.float32, tag="qf", name="qf")
                    cq = gpool.tile([128, S], dt.float32, tag="cq", name="cq")
                    fq = gpool.tile([128, S], dt.float32, tag="fq", name="fq")
                    iq = gpool.tile([128, S], dt.int32, tag="iq", name="iq")

                    for (startt, vect, w_t, qof) in (
                        (p1x, vecx, wx_t, None),
                        (p1y, vecy, wy_t, qf),
                    ):
                        st = startt[:, gcol : gcol + 1]
                        vt = vect[:, gcol : gcol + 1]
                        # raw sample coord, then exact reference mapping:
                        # g = coord/63*2-1 ; pix_pad = g*32 + 32.5
                        nc.vector.tensor_scalar(
                            out=cq[:], in0=trow_b, scalar1=vt, scalar2=st,
                            op0=op.mult, op1=op.add,
                        )
                        # g = (coord/63)*2 - 1 ; pix = (g+1)*32 - 0.5 ; +1 pad
                        nc.vector.tensor_scalar(
                            out=cq[:], in0=cq[:],
                            scalar1=float(np.float32(1.0 / 63.0)), scalar2=None,
                            op0=op.mult,
                        )
                        nc.vector.tensor_scalar(
                            out=cq[:], in0=cq[:], scalar1=2.0, scalar2=-1.0,
                            op0=op.mult, op1=op.add,
                        )
                        nc.vector.tensor_scalar(
                            out=cq[:], in0=cq[:], scalar1=1.0, scalar2=None,
                            op0=op.add,
                        )
                        nc.vector.tensor_scalar(
                            out=cq[:], in0=cq[:], scalar1=32.0, scalar2=-0.5,
                            op0=op.mult, op1=op.add,
                        )
                        nc.vector.tensor_scalar(
                            out=cq[:], in0=cq[:], scalar1=1.0, scalar2=None,
                            op0=op.add,
                        )
                        nc.vector.tensor_scalar(
                            out=cq[:], in0=cq[:], scalar1=0.0, scalar2=float(CLIPHI),
                            op0=op.max, op1=op.min,
                        )
                        # floor(cq): int cast rounds-to-nearest on HW
                        # (truncates in CoreSim) -> round then fix up, exact
                        # under either semantics.
                        nc.vector.tensor_copy(out=iq[:], in_=cq[:])
                        nc.vector.tensor_copy(out=fq[:], in_=iq[:])
                        nc.vector.tensor_tensor(
                            out=w_t[:], in0=cq[:], in1=fq[:], op=op.subtract
                        )
                        neg = gpool.tile([128, S], dt.float32, tag="neg", name="neg")
                        nc.vector.tensor_scalar(
                            out=neg[:], in0=w_t[:], scalar1=0.0, scalar2=None,
                            op0=op.is_lt,
                        )
                        nc.vector.tensor_tensor(
                            out=fq[:], in0=fq[:], in1=neg[:], op=op.subtract
                        )
                        nc.vector.tensor_tensor(
                            out=w_t[:], in0=cq[:], in1=fq[:], op=op.subtract
                        )
                        if qof is None:  # x pass: qf = xq
                            nc.vector.tensor_copy(out=qf[:], in_=fq[:])
                        else:  # y pass: qf = yq*66 + xq
                            nc.vector.scalar_tensor_tensor(
                                out=qf[:], in0=fq[:], scalar=66.0, in1=qf[:],
                                op0=op.mult, op1=op.add,
                            )
                    nc.vector.tensor_tensor(
                        out=wxy_t[:], in0=wx_t[:], in1=wy_t[:], op=op.mult
                    )
                    nc.vector.tensor_scalar(
                        out=qf[:], in0=qf[:], scalar1=0.0, scalar2=float(NQ - 1),
                        op0=op.max, op1=op.min,
                    )
                    nc.vector.tensor_copy(out=qi_t[:], in_=qf[:])

                    macc = apool.tile([128, C], dt.float32, tag="macc", name="macc")
                    nc.vector.memset(macc[:], -1e30)
                    for sc in range(NCH):
                        gtile = wpool.tile([128, SCH, 4 * C], TD, tag="gt", name="gt")
                        for s in range(SCH):
                            ws = sc * SCH + s
                            nc.gpsimd.indirect_dma_start(
                                out=gtile[:, s, :],
                                out_offset=None,
                                in_=tabs[n][:],
                                in_offset=bass.IndirectOffsetOnAxis(
                                    ap=qi_t[:, ws : ws + 1], axis=0
                                ),
                            )
                        acc = apool.tile([128, SCH, C], dt.float32, tag="acc", name="acc")
                        for s in range(SCH):
                            ws = sc * SCH + s
                            nc.vector.scalar_tensor_tensor(
                                out=acc[:, s, :],
                                in0=gtile[:, s, C : 2 * C],
                                scalar=wx_t[:, ws : ws + 1],
                                in1=gtile[:, s, 0:C],
                                op0=op.mult, op1=op.add,
                            )
                            nc.vector.scalar_tensor_tensor(
                                out=acc[:, s, :],
                                in0=gtile[:, s, 2 * C : 3 * C],
                                scalar=wy_t[:, ws : ws + 1],
                                in1=acc[:, s, :],
                                op0=op.mult, op1=op.add,
                            )
                            nc.vector.scalar_tensor_tensor(
                                out=acc[:, s, :],
                                in0=gtile[:, s, 3 * C : 4 * C],
                                scalar=wxy_t[:, ws : ws + 1],
                                in1=acc[:, s, :],
                                op0=op.mult, op1=op.add,
                            )
                        red = apool.tile([128, C], dt.float32, tag="red", name="red")
                        nc.vector.tensor_reduce(
                            out=red[:],
                            in_=acc[:].rearrange("p s c -> p c s"),
                            axis=mybir.AxisListType.X,
                            op=op.max,
                        )
                        nc.vector.tensor_tensor(
                            out=macc[:], in0=macc[:], in1=red[:], op=op.max
                        )
                    # transpose [128 pix, 256 c] -> two [128 c, 128 pix]
                    for ch in range(C // 128):
                        pt = pspool.tile([128, 128], dt.float32, tag="pt", name="pt")
                        nc.tensor.transpose(
                            out=pt[:],
                            in_=macc[:, ch * 128 : (ch + 1) * 128],
                            identity=ident[:],
                        )
                        ot = wpool.tile([128, 128], dt.float32, tag="ot", name="ot")
                        nc.scalar.copy(out=ot[:], in_=pt[:])
                        nc.sync.dma_start(
                            out=y_out[n, ch * 128 : (ch + 1) * 128,
                                      2 * col : 2 * col + 2, :],
                            in_=ot[:],
                        )

    nc.compile()
    return nc


_PROGRAM_CACHE = {}


def _get_program(table_dt_name=TABLE_DT):
    if table_dt_name not in _PROGRAM_CACHE:
        _PROGRAM_CACHE[table_dt_name] = build_program(table_dt_name)
    return _PROGRAM_CACHE[table_dt_name]


def make_in_maps(feat2, P1, P2, table_dt_name=TABLE_DT):
    np_dt = {"float32": np.float32, "float16": np.float16}[table_dt_name]
    F = _find_fundamental_host(P1, P2)
    tables = _build_tables(np.asarray(feat2, np.float32), np_dt)
    fmat = np.zeros((1, 16 * N), np.float32)
    for n in range(N):
        for i in range(3):
            f0, f1, f2 = F[n, i, 0], F[n, i, 1], F[n, i, 2]
            # split f1 = hi + lo with hi having <= 18 mantissa bits so that
            # hi*y and lo*y are exact for 6-bit integer y
            # split via masking the low 6 mantissa bits
            b = np.float32(f1).view(np.uint32)
            f1hi = np.uint32(b & np.uint32(0xFFFFFFC0)).view(np.float32)
            f1lo = np.float32(np.float64(f1) - np.float64(f1hi))
            fmat[0, 16 * n + 4 * i + 0] = f0
            fmat[0, 16 * n + 4 * i + 1] = f1hi
            fmat[0, 16 * n + 4 * i + 2] = f1lo
            fmat[0, 16 * n + 4 * i + 3] = f2
    fmat = np.broadcast_to(fmat, (128, 16 * N)).copy()
    p = np.arange(128)
    xw = np.broadcast_to((p % 64).astype(np.float32)[:, None], (128, NCOL)).copy()
    trow = np.broadcast_to(
        np.linspace(0.0, 1.0, S, dtype=np.float32).reshape(1, S), (128, S)
    ).copy()
    in_maps = []
    for k in range(NCORES):
        yh = np.zeros((128, NCOL), np.float32)
        for col in range(NCOL):
            yh[:, col] = 8 * k + 2 * col + p // 64
        in_maps.append(
            {
                "table0": tables[0],
                "table1": tables[1],
                "fmat": fmat,
                "xw": xw,
                "yh": yh,
                "trow": trow,
            }
        )
    return in_maps


def kernel(feat1, feat2, P1, P2, trace=False):
    from concourse.bass_utils import run_bass_kernel_spmd

    nc = _get_program(TABLE_DT)
    in_maps = make_in_maps(feat2, P1, P2, TABLE_DT)
    res = run_bass_kernel_spmd(
        nc, in_maps, core_ids=list(range(NCORES)), trace=trace
    )
    out = np.empty((N, C, H, W), np.float32)
    for k in range(NCORES):
        out[:, :, 8 * k : 8 * (k + 1), :] = res.results[k]["y"]
    if trace:
        kernel.last_results = res
    return out


# revision 16
# speedup vs baseline: 1.0802x; 1.0802x over previous
"""Trainium2 Bass kernel for nn_Epipolar (epipolar max-sampling).

Strategy
--------
out[n,c,h,w] = max_s bilinear(feat2[n,:,:,:], loc(s, n, h, w))

* Host: computes the 3x3 fundamental matrices F[n] (eigh/pinv of the tiny
  3x4 camera matrices, 24 input floats) and re-lays-out feat2 into a padded
  interleaved lookup table T[n][(y+1)*66+(x+1)] = [g, dx, dy, dxy] per
  channel, where g is the pixel value and dx/dy/dxy are forward differences.
  With that table a bilinear sample is g + wx*dx + wy*dy + wx*wy*dxy with a
  single gathered row per sample.
* Device (8 NeuronCores, data-parallel over output rows; core k owns output
  rows [8k, 8k+8) of both batch elements — outputs are disjoint, no
  collectives):
    - per-pixel epipolar line coefficients  l = F @ [x, y, 1]
    - border intersections, validity masks, stable pick-first-2 selection
    - 64 sample positions per pixel, bilinear weights + table indices
    - indirect-DMA gather of 1024-float table rows (128 pixels x 8 samples
      per chunk), fused scalar_tensor_tensor bilinear, running max
    - PE transpose [pixel, channel] -> [channel, pixel] and DMA out.
"""

import numpy as np

# ---------------------------------------------------------------------------
# Problem constants (hardcoded per contract)
N, C, H, W = 2, 256, 64, 64
S = 64
EPS = 1e-3
NCORES = 8
ROWS_PER_CORE = H // NCORES          # 8
PIX_PER_N = ROWS_PER_CORE * W        # 512 pixels per batch element per core
NCOL = PIX_PER_N // 128              # 4 partition-blocks of 128 pixels
GP = 66                              # padded grid is 66x66 (coords -1..64)
NQ = GP * GP                         # 4356 table rows
SCH = 8                              # samples per gather chunk
NCH = S // SCH                       # chunks per (n, col)
CLIPHI = np.float32(64.999996)       # just under 65, keeps x0 <= 64

TABLE_DT = "float32"                 # table dtype: float32 (safe) / float16


def _find_fundamental_host(P1, P2):
    """Exact replica of reference.find_fundamental, on jax CPU (matches the
    harness's own f32 eigh/pinv numerics)."""
    import jax

    with jax.default_device(jax.devices("cpu")[0]):
        import jax.numpy as jnp

        P1j = jnp.asarray(P1, jnp.float32)
        P2j = jnp.asarray(P2, jnp.float32)
        M = jnp.einsum("nij,nik->njk", P1j, P1j)
        _, V = jnp.linalg.eigh(M)
        Cc = V[..., 0]
        e2 = jnp.einsum("nij,nj->ni", P2j, Cc)
        z = jnp.zeros_like(e2[:, 0])
        skew = jnp.stack(
            [
                jnp.stack([z, -e2[:, 2], e2[:, 1]], -1),
                jnp.stack([e2[:, 2], z, -e2[:, 0]], -1),
                jnp.stack([-e2[:, 1], e2[:, 0], z], -1),
            ],
            -2,
        )
        F = skew @ P2j @ jnp.linalg.pinv(P1j)
        return np.asarray(F, np.float32)


def _build_tables(feat2, np_dt):
    """[N, NQ, 4*C] interleaved (g, dx, dy, dxy) over the padded 66x66 grid."""
    pad = np.zeros((N, H + 3, W + 3, C), np.float32)
    pad[:, 1 : H + 1, 1 : W + 1, :] = feat2.transpose(0, 2, 3, 1)
    g = pad[:, 0:GP, 0:GP, :]
    r = pad[:, 0:GP, 1 : GP + 1, :]
    d = pad[:, 1 : GP + 1, 0:GP, :]
    q = pad[:, 1 : GP + 1, 1 : GP + 1, :]
    T = np.concatenate([g, r - g, d - g, q - r - d + g], axis=-1)  # N,66,66,4C
    return np.ascontiguousarray(T.reshape(N, NQ, 4 * C).astype(np_dt))


# ---------------------------------------------------------------------------
# Bass program
# ---------------------------------------------------------------------------

def build_program(table_dt_name=TABLE_DT):
    import concourse.bacc as bacc
    import concourse.bass as bass
    import concourse.mybir as mybir
    import concourse.tile as tile
    from concourse.masks import make_identity

    dt = mybir.dt
    op = mybir.AluOpType
    TD = getattr(dt, table_dt_name)

    nc = bacc.Bacc("TRN2", target_bir_lowering=False, debug=False)

    tabs = [
        nc.dram_tensor(f"table{n}", [NQ, 4 * C], TD, kind="ExternalInput").ap()
        for n in range(N)
    ]
    fmat = nc.dram_tensor("fmat", [128, 16 * N], dt.float32, kind="ExternalInput").ap()
    xw_in = nc.dram_tensor("xw", [128, NCOL], dt.float32, kind="ExternalInput").ap()
    yh_in = nc.dram_tensor("yh", [128, NCOL], dt.float32, kind="ExternalInput").ap()
    trow_in = nc.dram_tensor("trow", [128, S], dt.float32, kind="ExternalInput").ap()
    cv_in = nc.dram_tensor("cvals", [128, 8], dt.float32, kind="ExternalInput").ap()
    y_out = nc.dram_tensor(
        "y", [N, C, ROWS_PER_CORE, W], dt.float32, kind="ExternalOutput"
    ).ap()

    with tile.TileContext(nc) as tc:
        with (
            tc.tile_pool(name="const", bufs=1) as cpool,
            tc.tile_pool(name="geo", bufs=1) as gpool,
            tc.tile_pool(name="pers", bufs=1) as ppool,
            tc.tile_pool(name="work", bufs=3) as wpool,
            tc.tile_pool(name="acc", bufs=3) as apool,
            tc.tile_pool(name="psum", bufs=2, space="PSUM") as pspool,
        ):
            # ---- constants to SBUF
            fm = cpool.tile([128, 16 * N], dt.float32, tag="fm", name="fm")
            xw = cpool.tile([128, NCOL], dt.float32, tag="xw", name="xw")
            yh = cpool.tile([128, NCOL], dt.float32, tag="yh", name="yh")
            trow = cpool.tile([128, S], dt.float32, tag="trow", name="trow")
            cv = cpool.tile([128, 8], dt.float32, tag="cv", name="cv")
            ident = cpool.tile([128, 128], dt.float32, tag="ident", name="ident")
            nc.sync.dma_start(out=fm[:], in_=fmat[:])
            nc.sync.dma_start(out=xw[:], in_=xw_in[:])
            nc.sync.dma_start(out=yh[:], in_=yh_in[:])
            nc.sync.dma_start(out=trow[:], in_=trow_in[:])
            nc.sync.dma_start(out=cv[:], in_=cv_in[:])
            make_identity(nc, ident[:])

            def F(n, i):
                return fm[:, 16 * n + i : 16 * n + i + 1]

            P8 = [128, N * NCOL]  # geometry tiles cover both batches

            def gt(tag):
                return gpool.tile(P8, dt.float32, tag=tag, name=tag)

            a_t, b_t, c_t = gt("a"), gt("b"), gt("c")
            tmp, tmp2, tmp3 = gt("tmp"), gt("tmp2"), gt("tmp3")

            # line coefficients per batch half, replicating XLA's einsum
            # rounding exactly: abc = fl(fma(F1, y, fl(F0*x)) + F2).
            # y is a small integer, so F1*y = F1hi*y + F1lo*y with both
            # products exact; the fma is then an exactly-rounded 3-term sum
            # via TwoSum.
            s_t, bb_t, u1_t, u2_t, u3_t, e_t2 = (
                gt("s_t"), gt("bb_t"), gt("u1_t"), gt("u2_t"), gt("u3_t"),
                gt("e_t2"),
            )
            for n in range(N):
                sl = slice(n * NCOL, (n + 1) * NCOL)
                for dst, r0 in ((a_t, 0), (b_t, 1), (c_t, 2)):
                    c0 = tmp[:, sl]
                    t1 = tmp2[:, sl]
                    t2 = tmp3[:, sl]
                    nc.vector.tensor_scalar(
                        out=c0, in0=xw[:], scalar1=F(n, 4 * r0 + 0),
                        scalar2=None, op0=op.mult,
                    )
                    nc.vector.tensor_scalar(
                        out=t1, in0=yh[:], scalar1=F(n, 4 * r0 + 1),
                        scalar2=None, op0=op.mult,
                    )
                    nc.vector.tensor_scalar(
                        out=t2, in0=yh[:], scalar1=F(n, 4 * r0 + 2),
                        scalar2=None, op0=op.mult,
                    )
                    s, bb, u1, u2, u3, e2s = (
                        s_t[:, sl], bb_t[:, sl], u1_t[:, sl], u2_t[:, sl],
                        u3_t[:, sl], e_t2[:, sl],
                    )
                    nc.vector.tensor_tensor(out=s, in0=t1, in1=c0, op=op.add)
                    nc.vector.tensor_tensor(out=bb, in0=s, in1=t1, op=op.subtract)
                    nc.vector.tensor_tensor(out=u1, in0=s, in1=bb, op=op.subtract)
                    nc.vector.tensor_tensor(out=u2, in0=t1, in1=u1, op=op.subtract)
                    nc.vector.tensor_tensor(out=u3, in0=c0, in1=bb, op=op.subtract)
                    nc.vector.tensor_tensor(out=e2s, in0=u2, in1=u3, op=op.add)
                    nc.vector.tensor_tensor(out=e2s, in0=e2s, in1=t2, op=op.add)
                    nc.vector.tensor_tensor(out=s, in0=s, in1=e2s, op=op.add)
                    nc.vector.tensor_scalar(
                        out=dst[:, sl], in0=s, scalar1=F(n, 4 * r0 + 3),
                        scalar2=None, op0=op.add,
                    )

            negc = gt("negc")
            nc.vector.tensor_scalar(
                out=negc[:], in0=c_t[:], scalar1=-1.0, scalar2=None, op0=op.mult
            )

            # db = sign(b)*max(|b|, EPS), true divide (matches reference)
            db_t, da_t = gt("db_t"), gt("da_t")
            for srct, dst in ((b_t, db_t), (a_t, da_t)):
                nc.scalar.activation(
                    out=tmp[:], in_=srct[:], func=mybir.ActivationFunctionType.Abs
                )
                nc.vector.tensor_scalar(
                    out=tmp[:], in0=tmp[:], scalar1=float(EPS), scalar2=None,
                    op0=op.max,
                )
                nc.scalar.activation(
                    out=tmp3[:], in_=srct[:], func=mybir.ActivationFunctionType.Sign
                )
                nc.vector.tensor_tensor(
                    out=dst[:], in0=tmp3[:], in1=tmp[:], op=op.mult
                )

            rdb, rda = gt("rdb"), gt("rda")
            nc.vector.reciprocal(out=rdb[:], in_=db_t[:])
            nc.vector.reciprocal(out=rda[:], in_=da_t[:])

            by1, by2, bx0, bx3 = gt("by1"), gt("by2"), gt("bx0"), gt("bx3")
            nc.vector.tensor_tensor(out=by1[:], in0=negc[:], in1=rdb[:], op=op.mult)
            nc.vector.scalar_tensor_tensor(
                out=tmp[:], in0=a_t[:], scalar=-63.0, in1=negc[:],
                op0=op.mult, op1=op.add,
            )
            nc.vector.tensor_tensor(out=by2[:], in0=tmp[:], in1=rdb[:], op=op.mult)
            nc.vector.tensor_tensor(out=bx0[:], in0=negc[:], in1=rda[:], op=op.mult)
            nc.vector.scalar_tensor_tensor(
                out=tmp[:], in0=b_t[:], scalar=-63.0, in1=negc[:],
                op0=op.mult, op1=op.add,
            )
            nc.vector.tensor_tensor(out=bx3[:], in0=tmp[:], in1=rda[:], op=op.mult)

            # validity masks of the four border intersections
            masks = [gt(f"m{i}") for i in range(4)]
            mspec = [
                (bx0, op.is_ge, op.is_lt),
                (by1, op.is_gt, op.is_le),
                (by2, op.is_ge, op.is_lt),
                (bx3, op.is_gt, op.is_le),
            ]
            for mi, (src, lo_op, hi_op) in zip(masks, mspec):
                nc.vector.tensor_scalar(
                    out=tmp[:], in0=src[:], scalar1=float(EPS), scalar2=None, op0=lo_op
                )
                nc.vector.tensor_scalar(
                    out=tmp2[:], in0=src[:], scalar1=float(63.0 - EPS), scalar2=None,
                    op0=hi_op,
                )
                nc.vector.tensor_tensor(
                    out=mi[:], in0=tmp[:], in1=tmp2[:], op=op.mult
                )

            nint = gt("nint")
            nc.vector.tensor_tensor(out=nint[:], in0=masks[0][:], in1=masks[1][:], op=op.add)
            nc.vector.tensor_tensor(out=nint[:], in0=nint[:], in1=masks[2][:], op=op.add)
            nc.vector.tensor_tensor(out=nint[:], in0=nint[:], in1=masks[3][:], op=op.add)
            valid2 = gt("valid2")
            nc.vector.tensor_scalar(
                out=valid2[:], in0=nint[:], scalar1=2.0, scalar2=None, op0=op.is_ge
            )

            # tmp_mask: where(valid2, mask, [1,1,0,0])
            tmask = [gt(f"tm{i}") for i in range(4)]
            for i in range(4):
                if i < 2:
                    nc.vector.tensor_scalar(
                        out=tmp[:], in0=masks[i][:], scalar1=-1.0, scalar2=None,
                        op0=op.add,
                    )
                    nc.vector.tensor_tensor(
                        out=tmp2[:], in0=tmp[:], in1=valid2[:], op=op.mult
                    )
                    nc.vector.tensor_scalar(
                        out=tmask[i][:], in0=tmp2[:], scalar1=1.0, scalar2=None,
                        op0=op.add,
                    )
                else:
                    nc.vector.tensor_tensor(
                        out=tmask[i][:], in0=masks[i][:], in1=valid2[:], op=op.mult
                    )

            # keys + two smallest (keys are distinct by construction)
            keys = [gt(f"k{i}") for i in range(4)]
            for i in range(4):
                nc.vector.tensor_scalar(
                    out=keys[i][:], in0=tmask[i][:], scalar1=-4.0,
                    scalar2=float(4 + i), op0=op.mult, op1=op.add,
                )
            kmin = gt("kmin")
            nc.vector.tensor_tensor(out=kmin[:], in0=keys[0][:], in1=keys[1][:], op=op.min)
            nc.vector.tensor_tensor(out=kmin[:], in0=kmin[:], in1=keys[2][:], op=op.min)
            nc.vector.tensor_tensor(out=kmin[:], in0=kmin[:], in1=keys[3][:], op=op.min)
            e_sel = [gt(f"e{i}") for i in range(4)]
            for i in range(4):
                nc.vector.tensor_tensor(
                    out=e_sel[i][:], in0=keys[i][:], in1=kmin[:], op=op.is_equal
                )
            kmin2 = gt("kmin2")
            k2t = [gt(f"k2t{i}") for i in range(4)]
            for i in range(4):
                nc.vector.scalar_tensor_tensor(
                    out=k2t[i][:], in0=e_sel[i][:], scalar=100.0, in1=keys[i][:],
                    op0=op.mult, op1=op.add,
                )
            nc.vector.tensor_tensor(out=kmin2[:], in0=k2t[0][:], in1=k2t[1][:], op=op.min)
            nc.vector.tensor_tensor(out=kmin2[:], in0=kmin2[:], in1=k2t[2][:], op=op.min)
            nc.vector.tensor_tensor(out=kmin2[:], in0=kmin2[:], in1=k2t[3][:], op=op.min)
            f_sel = [gt(f"f{i}") for i in range(4)]
            for i in range(4):
                nc.vector.tensor_tensor(
                    out=f_sel[i][:], in0=keys[i][:], in1=kmin2[:], op=op.is_equal
                )

            # selected endpoint coordinates
            # point coords: p0=(bx0,0) p1=(0,by1) p2=(63,by2) p3=(bx3,63)
            def sel_xy(sel, xo, yo):
                # x = s0*bx0 + s2*63 + s3*bx3
                nc.vector.tensor_tensor(out=xo[:], in0=sel[0][:], in1=bx0[:], op=op.mult)
                nc.vector.scalar_tensor_tensor(
                    out=xo[:], in0=sel[2][:], scalar=63.0, in1=xo[:],
                    op0=op.mult, op1=op.add,
                )
                nc.vector.tensor_tensor(out=tmp[:], in0=sel[3][:], in1=bx3[:], op=op.mult)
                nc.vector.tensor_tensor(out=xo[:], in0=xo[:], in1=tmp[:], op=op.add)
                # y = s1*by1 + s2*by2 + s3*63
                nc.vector.tensor_tensor(out=yo[:], in0=sel[1][:], in1=by1[:], op=op.mult)
                nc.vector.tensor_tensor(out=tmp[:], in0=sel[2][:], in1=by2[:], op=op.mult)
                nc.vector.tensor_tensor(out=yo[:], in0=yo[:], in1=tmp[:], op=op.add)
                nc.vector.scalar_tensor_tensor(
                    out=yo[:], in0=sel[3][:], scalar=63.0, in1=yo[:],
                    op0=op.mult, op1=op.add,
                )

            p1x, p1y, p2x, p2y = gt("p1x"), gt("p1y"), gt("p2x"), gt("p2y")
            sel_xy(e_sel, p1x, p1y)
            sel_xy(f_sel, p2x, p2y)

            # invalid pixels -> both endpoints at -10000 (exact select)
            inval = gpool.tile(P8, dt.uint8, tag="inval", name="inval")
            nc.vector.tensor_scalar(
                out=inval[:], in0=nint[:], scalar1=2.0, scalar2=None, op0=op.is_lt
            )
            neg1e4 = gt("neg1e4")
            nc.vector.memset(neg1e4[:], -10000.0)
            for t in (p1x, p1y, p2x, p2y):
                nc.vector.copy_predicated(out=t[:], mask=inval[:], data=neg1e4[:])
            vecx, vecy = gt("vecx"), gt("vecy")
            nc.vector.tensor_tensor(out=vecx[:], in0=p2x[:], in1=p1x[:], op=op.subtract)
            nc.vector.tensor_tensor(out=vecy[:], in0=p2y[:], in1=p1y[:], op=op.subtract)

            # ---- per (n, col): sample expansion, weights, indices,
            #      then immediately the gather + bilinear + max for that block
            trow_b = trow[:]
            for n in range(N):
                for col in range(NCOL):
                    gcol = n * NCOL + col
                    key = (n, col)
                    wx_t = ppool.tile([128, S], dt.float32, tag="wx", name="wx", bufs=2)
                    wy_t = ppool.tile([128, S], dt.float32, tag="wy", name="wy", bufs=2)
                    wxy_t = ppool.tile([128, S], dt.float32, tag="wxy", name="wxy", bufs=2)
                    qi_t = ppool.tile([128, S], dt.int32, tag="qi", name="qi", bufs=2)

                    qf = gpool.tile([128, S], dt.float32, tag="qf", name="qf")
                    cq = gpool.tile([128, S], dt.float32, tag="cq", name="cq")
                    fq = gpool.tile([128, S], dt.float32, tag="fq", name="fq")
                    iq = gpool.tile([128, S], dt.int32, tag="iq", name="iq")

                    for (startt, vect, w_t, qof) in (
                        (p1x, vecx, wx_t, None),
                        (p1y, vecy, wy_t, qf),
                    ):
                        st = startt[:, gcol : gcol + 1]
                        vt = vect[:, gcol : gcol + 1]
                        # raw sample coord, then exact reference mapping:
                        # g = coord/63*2-1 ; pix_pad = g*32 + 32.5
                        ID = mybir.ActivationFunctionType.Identity
                        nc.scalar.activation(
                            out=cq[:], in_=trow_b, func=ID, scale=vt, bias=st
                        )
                        # g = (coord/63)*2 - 1 ; pix = (g+1)*32 - 0.5 ; +1 pad
                        nc.scalar.activation(
                            out=cq[:], in_=cq[:], func=ID, scale=cv[:, 0:1],
                        )
                        nc.scalar.activation(
                            out=cq[:], in_=cq[:], func=ID, scale=cv[:, 1:2],
                            bias=cv[:, 2:3],
                        )
                        nc.scalar.activation(
                            out=cq[:], in_=cq[:], func=ID, bias=cv[:, 3:4],
                        )
                        nc.scalar.activation(
                            out=cq[:], in_=cq[:], func=ID, scale=cv[:, 4:5],
                            bias=cv[:, 5:6],
                        )
                        nc.scalar.activation(
                            out=cq[:], in_=cq[:], func=ID, bias=cv[:, 3:4],
                        )
                        nc.vector.tensor_scalar(
                            out=cq[:], in0=cq[:], scalar1=0.0, scalar2=float(CLIPHI),
                            op0=op.max, op1=op.min,
                        )
                        # floor(cq): int cast rounds-to-nearest on HW
                        # (truncates in CoreSim) -> round then fix up, exact
                        # under either semantics.
                        nc.vector.tensor_copy(out=iq[:], in_=cq[:])
                        nc.vector.tensor_copy(out=fq[:], in_=iq[:])
                        nc.vector.tensor_tensor(
                            out=w_t[:], in0=cq[:], in1=fq[:], op=op.subtract
                        )
                        neg = gpool.tile([128, S], dt.float32, tag="neg", name="neg")
                        nc.vector.tensor_scalar(
                            out=neg[:], in0=w_t[:], scalar1=0.0, scalar2=None,
                            op0=op.is_lt,
                        )
                        nc.vector.tensor_tensor(
                            out=fq[:], in0=fq[:], in1=neg[:], op=op.subtract
                        )
                        nc.vector.tensor_tensor(
                            out=w_t[:], in0=cq[:], in1=fq[:], op=op.subtract
                        )
                        if qof is None:  # x pass: qf = xq
                            nc.vector.tensor_copy(out=qf[:], in_=fq[:])
                        else:  # y pass: qf = yq*66 + xq
                            nc.vector.scalar_tensor_tensor(
                                out=qf[:], in0=fq[:], scalar=66.0, in1=qf[:],
                                op0=op.mult, op1=op.add,
                            )
                    nc.vector.tensor_tensor(
                        out=wxy_t[:], in0=wx_t[:], in1=wy_t[:], op=op.mult
                    )
                    nc.vector.tensor_scalar(
                        out=qf[:], in0=qf[:], scalar1=0.0, scalar2=float(NQ - 1),
                        op0=op.max, op1=op.min,
                    )
                    nc.vector.tensor_copy(out=qi_t[:], in_=qf[:])

                    macc = apool.tile([128, C], dt.float32, tag="macc", name="macc")
                    nc.vector.memset(macc[:], -1e30)
                    for sc in range(NCH):
                        gtile = wpool.tile([128, SCH, 4 * C], TD, tag="gt", name="gt")
                        for s in range(SCH):
                            ws = sc * SCH + s
                            nc.gpsimd.indirect_dma_start(
                                out=gtile[:, s, :],
                                out_offset=None,
                                in_=tabs[n][:],
                                in_offset=bass.IndirectOffsetOnAxis(
                                    ap=qi_t[:, ws : ws + 1], axis=0
                                ),
                            )
                        acc = apool.tile([128, SCH, C], dt.float32, tag="acc", name="acc")
                        for s in range(SCH):
                            ws = sc * SCH + s
                            nc.vector.scalar_tensor_tensor(
                                out=acc[:, s, :],
                                in0=gtile[:, s, C : 2 * C],
                                scalar=wx_t[:, ws : ws + 1],
                                in1=gtile[:, s, 0:C],
                                op0=op.mult, op1=op.add,
                            )
                            nc.vector.scalar_tensor_tensor(
                                out=acc[:, s, :],
                                in0=gtile[:, s, 2 * C : 3 * C],
                                scalar=wy_t[:, ws : ws + 1],
                                in1=acc[:, s, :],
                                op0=op.mult, op1=op.add,
                            )
                            nc.vector.scalar_tensor_tensor(
                                out=acc[:, s, :],
                                in0=gtile[:, s, 3 * C : 4 * C],
                                scalar=wxy_t[:, ws : ws + 1],
                                in1=acc[:, s, :],
                                op0=op.mult, op1=op.add,
                            )
                        # pairwise max tree over the SCH slots (reads each
                        # element once at level 1; beats tensor_reduce's
                        # ~1.5ns/elem)
                        half = SCH
                        while half > 1:
                            half //= 2
                            nc.vector.tensor_tensor(
                                out=acc[:, 0:half, :],
                                in0=acc[:, 0:half, :],
                                in1=acc[:, half : 2 * half, :],
                                op=op.max,
                            )
                        nc.vector.tensor_tensor(
                            out=macc[:], in0=macc[:], in1=acc[:, 0, :], op=op.max
                        )
                    # transpose [128 pix, 256 c] -> two [128 c, 128 pix]
                    for ch in range(C // 128):
                        pt = pspool.tile([128, 128], dt.float32, tag="pt", name="pt")
                        nc.tensor.transpose(
                            out=pt[:],
                            in_=macc[:, ch * 128 : (ch + 1) * 128],
                            identity=ident[:],
                        )
                        ot = wpool.tile([128, 128], dt.float32, tag="ot", name="ot")
                        nc.scalar.copy(out=ot[:], in_=pt[:])
                        nc.sync.dma_start(
                            out=y_out[n, ch * 128 : (ch + 1) * 128,
                                      2 * col : 2 * col + 2, :],
                            in_=ot[:],
                        )

    nc.compile()
    return nc


_PROGRAM_CACHE = {}


def _get_program(table_dt_name=TABLE_DT):
    if table_dt_name not in _PROGRAM_CACHE:
        _PROGRAM_CACHE[table_dt_name] = build_program(table_dt_name)
    return _PROGRAM_CACHE[table_dt_name]


def make_in_maps(feat2, P1, P2, table_dt_name=TABLE_DT):
    np_dt = {"float32": np.float32, "float16": np.float16}[table_dt_name]
    F = _find_fundamental_host(P1, P2)
    tables = _build_tables(np.asarray(feat2, np.float32), np_dt)
    fmat = np.zeros((1, 16 * N), np.float32)
    for n in range(N):
        for i in range(3):
            f0, f1, f2 = F[n, i, 0], F[n, i, 1], F[n, i, 2]
            # split f1 = hi + lo with hi having <= 18 mantissa bits so that
            # hi*y and lo*y are exact for 6-bit integer y
            # split via masking the low 6 mantissa bits
            b = np.float32(f1).view(np.uint32)
            f1hi = np.uint32(b & np.uint32(0xFFFFFFC0)).view(np.float32)
            f1lo = np.float32(np.float64(f1) - np.float64(f1hi))
            fmat[0, 16 * n + 4 * i + 0] = f0
            fmat[0, 16 * n + 4 * i + 1] = f1hi
            fmat[0, 16 * n + 4 * i + 2] = f1lo
            fmat[0, 16 * n + 4 * i + 3] = f2
    fmat = np.broadcast_to(fmat, (128, 16 * N)).copy()
    p = np.arange(128)
    xw = np.broadcast_to((p % 64).astype(np.float32)[:, None], (128, NCOL)).copy()
    trow = np.broadcast_to(
        np.linspace(0.0, 1.0, S, dtype=np.float32).reshape(1, S), (128, S)
    ).copy()
    cvals = np.broadcast_to(
        np.array([[1.0 / 63.0, 2.0, -1.0, 1.0, 32.0, -0.5, 0.0, 0.0]],
                 np.float32), (128, 8)
    ).copy()
    in_maps = []
    for k in range(NCORES):
        yh = np.zeros((128, NCOL), np.float32)
        for col in range(NCOL):
            yh[:, col] = 8 * k + 2 * col + p // 64
        in_maps.append(
            {
                "table0": tables[0],
                "table1": tables[1],
                "fmat": fmat,
                "xw": xw,
                "yh": yh,
                "trow": trow,
                "cvals": cvals,
            }
        )
    return in_maps


def kernel(feat1, feat2, P1, P2, trace=False):
    from concourse.bass_utils import run_bass_kernel_spmd

    nc = _get_program(TABLE_DT)
    in_maps = make_in_maps(feat2, P1, P2, TABLE_DT)
    res = run_bass_kernel_spmd(
        nc, in_maps, core_ids=list(range(NCORES)), trace=trace
    )
    out = np.empty((N, C, H, W), np.float32)
    for k in range(NCORES):
        out[:, :, 8 * k : 8 * (k + 1), :] = res.results[k]["y"]
    if trace:
        kernel.last_results = res
    return out
